# revision 8
# baseline (speedup 1.0000x reference)
"""BitLinear (BitNet b1.58) Trainium2 kernel, 8-core data-parallel. v3

Per core (4096 tokens, weight replicated, fed host-transposed as wT):
    q  = round(x*scale)  integers in [-127,127]   -> exact in bf16
    t  = clip(round(W/s),-1,1) in {-1,0,1}        -> exact in bf16
    out = (q @ t.T) * (absmax*s/127) per token, stored bf16.

Engine plan per 128-token tile (steady state ~4.0us, PE-bound):
    SP   : x DMA in
    DVE  : absmax reduce, 1/absmax, scl, coef, both 512-col output scales
    ACT  : xm = x*scl + MAGIC (fp32 magic-number rounding), PSUM drain
    Pool : q = xm - MAGIC -> bf16 cast (SBUF->SBUF)
    PE   : 8 bf16 transposes (q -> qT via identity matmul, bf16 PSUM)
           + 16 bf16 matmuls (qT_k stationary, ternary tT moving)
    ACT ring: out DMA (bf16)

The weight phase overlaps the x ramp: wT streams in while tiles 0-1 are
quantized; ternary quantization runs in 512-col halves so the first
matmuls start as soon as the first halves exist; tiles 0/1 consume
oh0-halves of all k before any oh1-half is needed.
"""

import numpy as np

import concourse.bass as bass
import concourse.mybir as mybir
from concourse import tile, masks
from concourse.bass_utils import run_bass_kernel_spmd

F32 = mybir.dt.float32
BF16 = mybir.dt.bfloat16

N_CORES = 8
B, S, D_IN, D_OUT = 4, 8192, 1024, 1024
TOKENS = B * S                     # 32768
TOK_PER_CORE = TOKENS // N_CORES   # 4096
TILES = TOK_PER_CORE // 128        # 32
KT = D_IN // 128                   # 8 contraction k-chunks

QMAX = 127.0
MAGIC = 12582912.0                     # 1.5 * 2**23 -> RNE integer rounding
THR = float(np.nextafter(np.float32(1.5), np.float32(0)))  # largest f32 < 1.5


def _split_multiwaits(nc):
    """walrus here encodes at most ONE sem wait per instruction; Tile's tail
    drain (and occasionally other insts) carry several.  Split extras into
    single-wait NOPs on the same engine, preserving order."""
    for f in nc.m.functions:
        for bb in f.blocks:
            insts = list(bb.instructions)
            if not any(
                i.sync_info and len(i.sync_info.on_wait) > 1 for i in insts
            ):
                continue
            new = []
            for ins in insts:
                si = ins.sync_info
                if si and len(si.on_wait) > 1:
                    waits = list(si.on_wait)
                    for j, w in enumerate(waits[:-1]):
                        nop = mybir.InstNoOp(
                            name=f"{ins.name}_wsp{j}", ins=[], outs=[]
                        )
                        nop.engine = ins.engine
                        nop.sync_info = mybir.SyncInfo(on_wait=[w], on_update=[])
                        new.append(nop)
                    ins.sync_info = mybir.SyncInfo(
                        on_wait=[waits[-1]], on_update=list(si.on_update)
                    )
                new.append(ins)
            bb.instructions = new


def build_program():
    nc = bass.Bass(trn_type="TRN2")
    x_d = nc.dram_tensor("x", [TOK_PER_CORE, D_IN], F32, kind="ExternalInput")
    w_d = nc.dram_tensor("wT", [D_IN, D_OUT], F32, kind="ExternalInput")
    o_d = nc.dram_tensor("out", [TOK_PER_CORE, D_OUT], BF16, kind="ExternalOutput")

    Copy = mybir.ActivationFunctionType.Copy
    Abs = mybir.ActivationFunctionType.Abs
    AX = mybir.AxisListType.X
    op = mybir.AluOpType

    with tile.TileContext(nc) as tc:
        from contextlib import ExitStack

        with ExitStack() as ctx:
            singles = ctx.enter_context(tc.tile_pool(name="singles", bufs=1))

            ident = singles.tile([128, 128], BF16)
            masks.make_identity(nc, ident[:])
            ones_col = singles.tile([128, 1], F32)
            nc.vector.memset(ones_col[:], 1.0)
            ones_row = singles.tile([1, 128], F32)
            nc.vector.memset(ones_row[:], 1.0)
            bc2 = singles.tile([128, 2], F32)    # [s, 1/s] broadcast to 128 parts
            s127_bc = singles.tile([128, 1], F32)  # s/127 broadcast

            tT = [singles.tile([128, D_OUT], BF16, name=f"tT{k}", tag=f"tT{k}") for k in range(KT)]

            xpool = ctx.enter_context(tc.tile_pool(name="xpool", bufs=8))
            xmpool = ctx.enter_context(tc.tile_pool(name="xmpool", bufs=3))
            qpool = ctx.enter_context(tc.tile_pool(name="qpool", bufs=3))
            qtpool = ctx.enter_context(tc.tile_pool(name="qtpool", bufs=6))
            outpool = ctx.enter_context(tc.tile_pool(name="outpool", bufs=3))
            smpool = ctx.enter_context(tc.tile_pool(name="smpool", bufs=12))
            psq = ctx.enter_context(tc.tile_pool(name="psq", bufs=2, space="PSUM"))
            pso = ctx.enter_context(tc.tile_pool(name="pso", bufs=4, space="PSUM"))
            psm = ctx.enter_context(tc.tile_pool(name="psm", bufs=2, space="PSUM"))

            live = {}

            def a_dma(n):
                """x tile DMA (SP) + per-token absmax/scale smalls (DVE)."""
                x_t = xpool.tile([128, D_IN], F32, tag="x")
                nc.sync.dma_start(x_t[:], x_d[n * 128:(n + 1) * 128, :])
                am = smpool.tile([128, 1], F32, tag="am")
                nc.vector.tensor_reduce(
                    am[:], x_t[:], axis=AX, op=op.max, apply_absolute_value=True
                )
                ram = smpool.tile([128, 1], F32, tag="ram")
                nc.vector.reciprocal(ram[:], am[:])
                scl = smpool.tile([128, 1], F32, tag="scl")
                nc.vector.tensor_scalar(scl[:], ram[:], QMAX, None, op0=op.mult)
                live[("x", n)] = x_t
                live[("am", n)] = am
                live[("scl", n)] = scl

            def a_quant(n):
                """ACT magic-round pass + Pool bf16 cast."""
                x_t = live.pop(("x", n))
                scl = live.pop(("scl", n))
                xm = xmpool.tile([128, D_IN], F32, tag="xm")
                nc.scalar.activation(xm[:], x_t[:], Copy, bias=MAGIC, scale=scl[:])
                q = qpool.tile([128, D_IN], BF16, tag="q")
                nc.gpsimd.tensor_scalar(q[:], xm[:], -MAGIC, None, op0=op.add)
                live[("q", n)] = q

            def a_trans(n):
                """PE bf16 transposes + ACT drain to SBUF."""
                q = live.pop(("q", n))
                ps_q = psq.tile([128, D_IN], BF16, tag="ps_q")
                for k in range(KT):
                    nc.tensor.transpose(
                        ps_q[:, k * 128:(k + 1) * 128],
                        q[:, k * 128:(k + 1) * 128],
                        ident[:],
                    )
                qT = qtpool.tile([128, D_IN], BF16, tag="qT")
                nc.scalar.copy(qT[:], ps_q[:])
                live[("qT", n)] = qT

            def b_coef(n):
                am = live.pop(("am", n))
                coef = smpool.tile([128, 1], F32, tag="coef")
                nc.vector.tensor_scalar(coef[:], am[:], s127_bc[:], None, op0=op.mult)
                live[("coef", n)] = coef

            def b_mm_half(n, oh, qT=None):
                qT = qT if qT is not None else live[("qT", n)]
                ps = pso.tile([128, 512], F32, tag="ps")
                for k in range(KT):
                    nc.tensor.matmul(
                        ps[:], qT[:, k * 128:(k + 1) * 128],
                        tT[k][:, oh * 512:(oh + 1) * 512],
                        start=(k == 0), stop=(k == KT - 1),
                    )
                live[("ps", n, oh)] = ps

            def b_out_half(n, oh, out_sb):
                ps = live.pop(("ps", n, oh))
                coef = live[("coef", n)]
                nc.vector.tensor_scalar(
                    out_sb[:, oh * 512:(oh + 1) * 512], ps[:], coef[:], None,
                    op0=op.mult,
                )

            def b(n):
                """full tile: coef, both matmul halves, scales, out DMA."""
                b_coef(n)
                qT = live.pop(("qT", n))
                b_mm_half(n, 0, qT)
                b_mm_half(n, 1, qT)
                out_sb = outpool.tile([128, D_OUT], BF16, tag="osb")
                b_out_half(n, 0, out_sb)
                b_out_half(n, 1, out_sb)
                live.pop(("coef", n))
                nc.scalar.dma_start(o_d[n * 128:(n + 1) * 128, :], out_sb[:])

            # ---------------- weight phase + x ramp ------------------------
            with (
                tc.tile_pool(name="wpool", bufs=1) as wpool,
                tc.tile_pool(name="wabs", bufs=2) as wabs_pool,
                tc.tile_pool(name="ypool", bufs=4) as ypool,
            ):
                # SP DMA order = HBM priority: x0,x1 for the ramp, then all
                # of wT (gates the matmuls), then the rest of x.
                a_dma(0)
                a_dma(1)
                w_t = [wpool.tile([128, D_OUT], F32, name=f"w{k}", tag=f"w{k}") for k in range(KT)]
                for k in range(KT):
                    nc.sync.dma_start(w_t[k][:], w_d[k * 128:(k + 1) * 128, :])
                a_dma(2)
                a_dma(3)
                a_dma(4)
                a_dma(5)

                # ramp tiles 0,1 quant+transpose while wT streams in
                a_quant(0)
                a_trans(0)
                a_quant(1)
                a_trans(1)

                # |wT| chunk sums (ACT even / DVE odd), then mean -> s, 1/s
                colsum = wpool.tile([128, KT], F32)
                for k in range(KT):
                    if k % 2 == 0:
                        wabs = wabs_pool.tile([128, D_OUT], F32, tag="wabs")
                        nc.scalar.activation(
                            wabs[:], w_t[k][:], Abs, accum_out=colsum[:, k:k + 1]
                        )
                    else:
                        nc.vector.tensor_reduce(
                            colsum[:, k:k + 1], w_t[k][:], axis=AX, op=op.add,
                            apply_absolute_value=True,
                        )
                colsum2 = wpool.tile([128, 1], F32)
                nc.vector.tensor_reduce(colsum2[:], colsum[:], axis=AX, op=op.add)

                ps_m1 = psm.tile([1, 2], F32, name="ps_m1", tag="ps_m")
                nc.tensor.matmul(ps_m1[0:1, 0:1], ones_col[:], colsum2[:])
                pair = wpool.tile([1, 2], F32)
                nc.scalar.activation(pair[:, 0:1], ps_m1[0:1, 0:1], Copy, scale=1.0 / (D_OUT * D_IN))
                nc.vector.reciprocal(pair[:, 1:2], pair[:, 0:1])
                ps_m2 = psm.tile([128, 2], F32, name="ps_m2", tag="ps_m")
                nc.tensor.matmul(ps_m2[:], ones_row[:], pair[:])
                nc.scalar.copy(bc2[:], ps_m2[:])
                nc.vector.tensor_scalar(s127_bc[:], bc2[:, 0:1], 1.0 / QMAX, None, op0=op.mult)

                # ternary-quantize wT in 512-col halves (oh-major) so the
                # first matmuls can start before the oh1 halves exist.
                def wq_half(k, oh):
                    sl = slice(oh * 512, (oh + 1) * 512)
                    y0 = ypool.tile([128, 512], F32, tag="y0")
                    nc.scalar.activation(y0[:], w_t[k][:, sl], Copy, scale=bc2[:, 1:2])
                    y1 = ypool.tile([128, 512], F32, tag="y1")
                    nc.vector.tensor_scalar(y1[:], y0[:], THR, -THR, op0=op.min, op1=op.max)
                    nc.vector.tensor_scalar(
                        tT[k][:, sl], y1[:], MAGIC, -MAGIC, op0=op.add, op1=op.add
                    )

                for k in range(KT):
                    wq_half(k, 0)
                # tiles 0,1: consume oh0 halves while oh1 halves are produced
                # (all oh1 writes are EMITTED before the oh1 matmuls so Tile
                # sees the dependency; execution still overlaps per-k.)
                b_coef(0)
                b_coef(1)
                qT0 = live.pop(("qT", 0))
                qT1 = live.pop(("qT", 1))
                b_mm_half(0, 0, qT0)
                wq_half(0, 1)
                wq_half(1, 1)
                a_quant(2)
                b_mm_half(1, 0, qT1)
                wq_half(2, 1)
                wq_half(3, 1)
                a_trans(2)
                wq_half(4, 1)
                wq_half(5, 1)
                a_quant(3)
                wq_half(6, 1)
                wq_half(7, 1)
                b_mm_half(0, 1, qT0)
                b_mm_half(1, 1, qT1)
                a_trans(3)

                osb0 = outpool.tile([128, D_OUT], BF16, tag="osb")
                b_out_half(0, 0, osb0)
                b_out_half(0, 1, osb0)
                live.pop(("coef", 0))
                nc.scalar.dma_start(o_d[0:128, :], osb0[:])
                osb1 = outpool.tile([128, D_OUT], BF16, tag="osb")
                b_out_half(1, 0, osb1)
                b_out_half(1, 1, osb1)
                live.pop(("coef", 1))
                nc.scalar.dma_start(o_d[128:256, :], osb1[:])

            a_quant(4)
            a_trans(4)

            for n in range(2, TILES):
                if n + 4 < TILES:
                    a_dma(n + 4)
                if n + 3 < TILES:
                    a_quant(n + 3)
                b(n)
                if n + 3 < TILES:
                    a_trans(n + 3)

    _split_multiwaits(nc)
    return nc


_NC_CACHE = None


def _get_nc():
    global _NC_CACHE
    if _NC_CACHE is None:
        _NC_CACHE = build_program()
    return _NC_CACHE


def kernel(x: np.ndarray, weight: np.ndarray, trace: bool = False):
    assert x.shape == (B, S, D_IN) and weight.shape == (D_OUT, D_IN)
    nc = _get_nc()
    xf = np.ascontiguousarray(x.reshape(TOKENS, D_IN), dtype=np.float32)
    wT = np.ascontiguousarray(weight.astype(np.float32, copy=False).T)
    in_maps = [
        {
            "x": xf[c * TOK_PER_CORE:(c + 1) * TOK_PER_CORE],
            "wT": wT,
        }
        for c in range(N_CORES)
    ]
    res = run_bass_kernel_spmd(nc, in_maps, core_ids=list(range(N_CORES)), trace=trace)
    kernel.last_results = res
    out = np.concatenate(
        [np.asarray(res.results[c]["out"]).astype(np.float32) for c in range(N_CORES)],
        axis=0,
    )
    return out.reshape(B, S, D_OUT)


kernel.last_results = None


# revision 10
# speedup vs baseline: 3.1351x; 3.1351x over previous
"""BitLinear (BitNet b1.58) Trainium2 kernel, 8-core data-parallel. v3

Per core (4096 tokens, weight replicated, fed host-transposed as wT):
    q  = round(x*scale)  integers in [-127,127]   -> exact in bf16
    t  = clip(round(W/s),-1,1) in {-1,0,1}        -> exact in bf16
    out = (q @ t.T) * (absmax*s/127) per token, stored bf16.

Engine plan per 128-token tile (steady state ~4.0us, PE-bound):
    SP   : x DMA in
    DVE  : absmax reduce, 1/absmax, scl, coef, both 512-col output scales
    ACT  : xm = x*scl + MAGIC (fp32 magic-number rounding), PSUM drain
    Pool : q = xm - MAGIC -> bf16 cast (SBUF->SBUF)
    PE   : 8 bf16 transposes (q -> qT via identity matmul, bf16 PSUM)
           + 16 bf16 matmuls (qT_k stationary, ternary tT moving)
    ACT ring: out DMA (bf16)

The weight phase overlaps the x ramp: wT streams in while tiles 0-1 are
quantized; ternary quantization runs in 512-col halves so the first
matmuls start as soon as the first halves exist; tiles 0/1 consume
oh0-halves of all k before any oh1-half is needed.
"""

import numpy as np

import concourse.bass as bass
import concourse.mybir as mybir
from concourse import tile, masks
from concourse.bass_utils import run_bass_kernel_spmd

F32 = mybir.dt.float32
BF16 = mybir.dt.bfloat16

N_CORES = 8
B, S, D_IN, D_OUT = 4, 8192, 1024, 1024
TOKENS = B * S                     # 32768
TOK_PER_CORE = TOKENS // N_CORES   # 4096
TILES = TOK_PER_CORE // 128        # 32
KT = D_IN // 128                   # 8 contraction k-chunks

QMAX = 127.0
MAGIC = 12582912.0                     # 1.5 * 2**23 -> RNE integer rounding
THR = float(np.nextafter(np.float32(1.5), np.float32(0)))  # largest f32 < 1.5


def _split_multiwaits(nc):
    """walrus here encodes at most ONE sem wait per instruction; Tile's tail
    drain (and occasionally other insts) carry several.  Split extras into
    single-wait NOPs on the same engine, preserving order."""
    for f in nc.m.functions:
        for bb in f.blocks:
            insts = list(bb.instructions)
            if not any(
                i.sync_info and len(i.sync_info.on_wait) > 1 for i in insts
            ):
                continue
            new = []
            for ins in insts:
                si = ins.sync_info
                if si and len(si.on_wait) > 1:
                    waits = list(si.on_wait)
                    for j, w in enumerate(waits[:-1]):
                        nop = mybir.InstNoOp(
                            name=f"{ins.name}_wsp{j}", ins=[], outs=[]
                        )
                        nop.engine = ins.engine
                        nop.sync_info = mybir.SyncInfo(on_wait=[w], on_update=[])
                        new.append(nop)
                    ins.sync_info = mybir.SyncInfo(
                        on_wait=[waits[-1]], on_update=list(si.on_update)
                    )
                new.append(ins)
            bb.instructions = new


def build_program():
    nc = bass.Bass(trn_type="TRN2")
    x_d = nc.dram_tensor("x", [TOK_PER_CORE, D_IN], F32, kind="ExternalInput")
    w_d = nc.dram_tensor("wT", [D_IN, D_OUT], F32, kind="ExternalInput")
    o_d = nc.dram_tensor("out", [TOK_PER_CORE, D_OUT], BF16, kind="ExternalOutput")

    Copy = mybir.ActivationFunctionType.Copy
    Abs = mybir.ActivationFunctionType.Abs
    AX = mybir.AxisListType.X
    op = mybir.AluOpType

    with tile.TileContext(nc) as tc:
        from contextlib import ExitStack

        with ExitStack() as ctx:
            singles = ctx.enter_context(tc.tile_pool(name="singles", bufs=1))

            ident = singles.tile([128, 128], BF16)
            masks.make_identity(nc, ident[:])
            ones_col = singles.tile([128, 1], F32)
            nc.vector.memset(ones_col[:], 1.0)
            ones_row = singles.tile([1, 128], F32)
            nc.vector.memset(ones_row[:], 1.0)
            bc2 = singles.tile([128, 2], F32)    # [s, 1/s] broadcast to 128 parts
            s127_bc = singles.tile([128, 1], F32)  # s/127 broadcast

            tT = [singles.tile([128, D_OUT], BF16, name=f"tT{k}", tag=f"tT{k}") for k in range(KT)]

            xpool = ctx.enter_context(tc.tile_pool(name="xpool", bufs=8))
            xmpool = ctx.enter_context(tc.tile_pool(name="xmpool", bufs=3))
            qpool = ctx.enter_context(tc.tile_pool(name="qpool", bufs=3))
            qtpool = ctx.enter_context(tc.tile_pool(name="qtpool", bufs=6))
            outpool = ctx.enter_context(tc.tile_pool(name="outpool", bufs=3))
            smpool = ctx.enter_context(tc.tile_pool(name="smpool", bufs=12))
            psq = ctx.enter_context(tc.tile_pool(name="psq", bufs=2, space="PSUM"))
            pso = ctx.enter_context(tc.tile_pool(name="pso", bufs=4, space="PSUM"))
            psm = ctx.enter_context(tc.tile_pool(name="psm", bufs=2, space="PSUM"))

            live = {}

            def a_dma(n):
                """x tile DMA (SP) + per-token absmax/scale smalls (DVE)."""
                x_t = xpool.tile([128, D_IN], F32, tag="x")
                nc.sync.dma_start(x_t[:], x_d[n * 128:(n + 1) * 128, :])
                am = smpool.tile([128, 1], F32, tag="am")
                nc.vector.tensor_reduce(
                    am[:], x_t[:], axis=AX, op=op.max, apply_absolute_value=True
                )
                ram = smpool.tile([128, 1], F32, tag="ram")
                nc.vector.reciprocal(ram[:], am[:])
                scl = smpool.tile([128, 1], F32, tag="scl")
                nc.vector.tensor_scalar(scl[:], ram[:], QMAX, None, op0=op.mult)
                live[("x", n)] = x_t
                live[("am", n)] = am
                live[("scl", n)] = scl

            def a_quant(n):
                """ACT magic-round pass + DVE bf16 cast.  (GPSIMD measured
                14.7us per 1024-elem op on HW - never offload bulk to it.)"""
                x_t = live.pop(("x", n))
                scl = live.pop(("scl", n))
                xm = xmpool.tile([128, D_IN], F32, tag="xm")
                nc.scalar.activation(xm[:], x_t[:], Copy, bias=MAGIC, scale=scl[:])
                q = qpool.tile([128, D_IN], BF16, tag="q")
                nc.vector.tensor_scalar(q[:], xm[:], -MAGIC, None, op0=op.add)
                live[("q", n)] = q

            def a_trans(n):
                """PE bf16 transposes + ACT drain to SBUF."""
                q = live.pop(("q", n))
                ps_q = psq.tile([128, D_IN], BF16, tag="ps_q")
                for k in range(KT):
                    nc.tensor.transpose(
                        ps_q[:, k * 128:(k + 1) * 128],
                        q[:, k * 128:(k + 1) * 128],
                        ident[:],
                    )
                qT = qtpool.tile([128, D_IN], BF16, tag="qT")
                nc.scalar.copy(qT[:], ps_q[:])
                live[("qT", n)] = qT

            def b_coef(n):
                am = live.pop(("am", n))
                coef = smpool.tile([128, 1], F32, tag="coef")
                nc.vector.tensor_scalar(coef[:], am[:], s127_bc[:], None, op0=op.mult)
                live[("coef", n)] = coef

            def b_mm_half(n, oh, qT=None):
                qT = qT if qT is not None else live[("qT", n)]
                ps = pso.tile([128, 512], F32, tag="ps")
                for k in range(KT):
                    nc.tensor.matmul(
                        ps[:], qT[:, k * 128:(k + 1) * 128],
                        tT[k][:, oh * 512:(oh + 1) * 512],
                        start=(k == 0), stop=(k == KT - 1),
                    )
                live[("ps", n, oh)] = ps

            def b_out_half(n, oh, out_sb):
                ps = live.pop(("ps", n, oh))
                coef = live[("coef", n)]
                if oh == 0:
                    nc.scalar.activation(
                        out_sb[:, 0:512], ps[:], Copy, scale=coef[:]
                    )
                else:
                    nc.vector.tensor_scalar(
                        out_sb[:, 512:1024], ps[:], coef[:], None, op0=op.mult
                    )

            def b(n):
                """full tile: coef, both matmul halves, scales, out DMA."""
                b_coef(n)
                qT = live.pop(("qT", n))
                b_mm_half(n, 0, qT)
                b_mm_half(n, 1, qT)
                out_sb = outpool.tile([128, D_OUT], BF16, tag="osb")
                b_out_half(n, 0, out_sb)
                b_out_half(n, 1, out_sb)
                live.pop(("coef", n))
                nc.scalar.dma_start(o_d[n * 128:(n + 1) * 128, :], out_sb[:])

            # ---------------- weight phase + x ramp ------------------------
            with (
                tc.tile_pool(name="wpool", bufs=1) as wpool,
                tc.tile_pool(name="wabs", bufs=2) as wabs_pool,
                tc.tile_pool(name="ypool", bufs=4) as ypool,
            ):
                # SP DMA order = HBM priority: x0,x1 for the ramp, then all
                # of wT (gates the matmuls), then the rest of x.
                a_dma(0)
                a_dma(1)
                w_t = [wpool.tile([128, D_OUT], F32, name=f"w{k}", tag=f"w{k}") for k in range(KT)]
                for k in range(KT):
                    nc.sync.dma_start(w_t[k][:], w_d[k * 128:(k + 1) * 128, :])
                a_dma(2)
                a_dma(3)
                a_dma(4)
                a_dma(5)

                # ramp tiles 0,1 quant+transpose while wT streams in
                a_quant(0)
                a_trans(0)
                a_quant(1)
                a_trans(1)

                # |wT| chunk sums (ACT even / DVE odd), then mean -> s, 1/s
                colsum = wpool.tile([128, KT], F32)
                for k in range(KT):
                    if k % 2 == 0:
                        wabs = wabs_pool.tile([128, D_OUT], F32, tag="wabs")
                        nc.scalar.activation(
                            wabs[:], w_t[k][:], Abs, accum_out=colsum[:, k:k + 1]
                        )
                    else:
                        nc.vector.tensor_reduce(
                            colsum[:, k:k + 1], w_t[k][:], axis=AX, op=op.add,
                            apply_absolute_value=True,
                        )
                colsum2 = wpool.tile([128, 1], F32)
                nc.vector.tensor_reduce(colsum2[:], colsum[:], axis=AX, op=op.add)

                ps_m1 = psm.tile([1, 2], F32, name="ps_m1", tag="ps_m")
                nc.tensor.matmul(ps_m1[0:1, 0:1], ones_col[:], colsum2[:])
                pair = wpool.tile([1, 2], F32)
                nc.scalar.activation(pair[:, 0:1], ps_m1[0:1, 0:1], Copy, scale=1.0 / (D_OUT * D_IN))
                nc.vector.reciprocal(pair[:, 1:2], pair[:, 0:1])
                ps_m2 = psm.tile([128, 2], F32, name="ps_m2", tag="ps_m")
                nc.tensor.matmul(ps_m2[:], ones_row[:], pair[:])
                nc.scalar.copy(bc2[:], ps_m2[:])
                nc.vector.tensor_scalar(s127_bc[:], bc2[:, 0:1], 1.0 / QMAX, None, op0=op.mult)

                # ternary-quantize wT in 512-col halves (oh-major) so the
                # first matmuls can start before the oh1 halves exist.
                def wq_half(k, oh):
                    sl = slice(oh * 512, (oh + 1) * 512)
                    y0 = ypool.tile([128, 512], F32, tag="y0")
                    nc.scalar.activation(y0[:], w_t[k][:, sl], Copy, scale=bc2[:, 1:2])
                    y1 = ypool.tile([128, 512], F32, tag="y1")
                    nc.vector.tensor_scalar(y1[:], y0[:], THR, -THR, op0=op.min, op1=op.max)
                    nc.vector.tensor_scalar(
                        tT[k][:, sl], y1[:], MAGIC, -MAGIC, op0=op.add, op1=op.add
                    )

                for k in range(KT):
                    wq_half(k, 0)
                # tiles 0,1: consume oh0 halves while oh1 halves are produced
                # (all oh1 writes are EMITTED before the oh1 matmuls so Tile
                # sees the dependency; execution still overlaps per-k.)
                b_coef(0)
                b_coef(1)
                qT0 = live.pop(("qT", 0))
                qT1 = live.pop(("qT", 1))
                b_mm_half(0, 0, qT0)
                wq_half(0, 1)
                wq_half(1, 1)
                a_quant(2)
                b_mm_half(1, 0, qT1)
                wq_half(2, 1)
                wq_half(3, 1)
                a_trans(2)
                wq_half(4, 1)
                wq_half(5, 1)
                a_quant(3)
                wq_half(6, 1)
                wq_half(7, 1)
                b_mm_half(0, 1, qT0)
                b_mm_half(1, 1, qT1)
                a_trans(3)

                osb0 = outpool.tile([128, D_OUT], BF16, tag="osb")
                b_out_half(0, 0, osb0)
                b_out_half(0, 1, osb0)
                live.pop(("coef", 0))
                nc.scalar.dma_start(o_d[0:128, :], osb0[:])
                osb1 = outpool.tile([128, D_OUT], BF16, tag="osb")
                b_out_half(1, 0, osb1)
                b_out_half(1, 1, osb1)
                live.pop(("coef", 1))
                nc.scalar.dma_start(o_d[128:256, :], osb1[:])

            a_quant(4)
            a_trans(4)

            for n in range(2, TILES):
                if n + 4 < TILES:
                    a_dma(n + 4)
                if n + 3 < TILES:
                    a_quant(n + 3)
                b(n)
                if n + 3 < TILES:
                    a_trans(n + 3)

    _split_multiwaits(nc)
    return nc


_NC_CACHE = None


def _get_nc():
    global _NC_CACHE
    if _NC_CACHE is None:
        _NC_CACHE = build_program()
    return _NC_CACHE


def kernel(x: np.ndarray, weight: np.ndarray, trace: bool = False):
    assert x.shape == (B, S, D_IN) and weight.shape == (D_OUT, D_IN)
    nc = _get_nc()
    xf = np.ascontiguousarray(x.reshape(TOKENS, D_IN), dtype=np.float32)
    wT = np.ascontiguousarray(weight.astype(np.float32, copy=False).T)
    in_maps = [
        {
            "x": xf[c * TOK_PER_CORE:(c + 1) * TOK_PER_CORE],
            "wT": wT,
        }
        for c in range(N_CORES)
    ]
    res = run_bass_kernel_spmd(nc, in_maps, core_ids=list(range(N_CORES)), trace=trace)
    kernel.last_results = res
    out = np.concatenate(
        [np.asarray(res.results[c]["out"]).astype(np.float32) for c in range(N_CORES)],
        axis=0,
    )
    return out.reshape(B, S, D_OUT)


kernel.last_results = None


# revision 17
# speedup vs baseline: 3.1425x; 1.0023x over previous
"""BitLinear (BitNet b1.58) Trainium2 kernel, 8-core data-parallel. v3

Per core (4096 tokens, weight replicated, fed host-transposed as wT):
    q  = round(x*scale)  integers in [-127,127]   -> exact in bf16
    t  = clip(round(W/s),-1,1) in {-1,0,1}        -> exact in bf16
    out = (q @ t.T) * (absmax*s/127) per token, stored bf16.

Engine plan per 128-token tile (steady state ~4.0us, PE-bound):
    SP   : x DMA in
    DVE  : absmax reduce, 1/absmax, scl, coef, both 512-col output scales
    ACT  : xm = x*scl + MAGIC (fp32 magic-number rounding), PSUM drain
    Pool : q = xm - MAGIC -> bf16 cast (SBUF->SBUF)
    PE   : 8 bf16 transposes (q -> qT via identity matmul, bf16 PSUM)
           + 16 bf16 matmuls (qT_k stationary, ternary tT moving)
    ACT ring: out DMA (bf16)

The weight phase overlaps the x ramp: wT streams in while tiles 0-1 are
quantized; ternary quantization runs in 512-col halves so the first
matmuls start as soon as the first halves exist; tiles 0/1 consume
oh0-halves of all k before any oh1-half is needed.
"""

import numpy as np

import concourse.bass as bass
import concourse.mybir as mybir
from concourse import tile, masks
from concourse.bass_utils import run_bass_kernel_spmd

F32 = mybir.dt.float32
BF16 = mybir.dt.bfloat16
FP16 = mybir.dt.float16

N_CORES = 8
B, S, D_IN, D_OUT = 4, 8192, 1024, 1024
TOKENS = B * S                     # 32768
TOK_PER_CORE = TOKENS // N_CORES   # 4096
TILES = TOK_PER_CORE // 128        # 32
KT = D_IN // 128                   # 8 contraction k-chunks

QMAX = 127.0
MAGIC = 12582912.0                     # 1.5 * 2**23 -> RNE integer rounding (fp32)
MAGIC16 = 1536.0                       # 1.5 * 2**10  -> RNE rounding via fp16 cast
THR = float(np.nextafter(np.float32(1.5), np.float32(0)))  # largest f32 < 1.5


def _split_multiwaits(nc):
    """walrus here encodes at most ONE sem wait per instruction; Tile's tail
    drain (and occasionally other insts) carry several.  Split extras into
    single-wait NOPs on the same engine, preserving order."""
    for f in nc.m.functions:
        for bb in f.blocks:
            insts = list(bb.instructions)
            if not any(
                i.sync_info and len(i.sync_info.on_wait) > 1 for i in insts
            ):
                continue
            new = []
            for ins in insts:
                si = ins.sync_info
                if si and len(si.on_wait) > 1:
                    waits = list(si.on_wait)
                    for j, w in enumerate(waits[:-1]):
                        nop = mybir.InstNoOp(
                            name=f"{ins.name}_wsp{j}", ins=[], outs=[]
                        )
                        nop.engine = ins.engine
                        nop.sync_info = mybir.SyncInfo(on_wait=[w], on_update=[])
                        new.append(nop)
                    ins.sync_info = mybir.SyncInfo(
                        on_wait=[waits[-1]], on_update=list(si.on_update)
                    )
                new.append(ins)
            bb.instructions = new


def build_program():
    nc = bass.Bass(trn_type="TRN2")
    x_d = nc.dram_tensor("x", [TOK_PER_CORE, D_IN], F32, kind="ExternalInput")
    w_d = nc.dram_tensor("wT", [D_IN, D_OUT], F32, kind="ExternalInput")
    o_d = nc.dram_tensor("out", [TOK_PER_CORE, D_OUT], BF16, kind="ExternalOutput")

    Copy = mybir.ActivationFunctionType.Copy
    Abs = mybir.ActivationFunctionType.Abs
    AX = mybir.AxisListType.X
    op = mybir.AluOpType

    with tile.TileContext(nc) as tc:
        from contextlib import ExitStack

        with ExitStack() as ctx:
            singles = ctx.enter_context(tc.tile_pool(name="singles", bufs=1))

            ident = singles.tile([128, 128], FP16)
            masks.make_identity(nc, ident[:])
            ones_col = singles.tile([128, 1], F32)
            nc.vector.memset(ones_col[:], 1.0)
            ones_row = singles.tile([1, 128], F32)
            nc.vector.memset(ones_row[:], 1.0)
            bc2 = singles.tile([128, 2], F32)    # [s, 1/s] broadcast to 128 parts
            s127_bc = singles.tile([128, 1], F32)  # s/127 broadcast

            tT = [singles.tile([128, D_OUT], BF16, name=f"tT{k}", tag=f"tT{k}") for k in range(KT)]

            xpool = ctx.enter_context(tc.tile_pool(name="xpool", bufs=8))
            xmpool = ctx.enter_context(tc.tile_pool(name="xmpool", bufs=3))
            qtpool = ctx.enter_context(tc.tile_pool(name="qtpool", bufs=6))
            outpool = ctx.enter_context(tc.tile_pool(name="outpool", bufs=3))
            smpool = ctx.enter_context(tc.tile_pool(name="smpool", bufs=12))
            psq = ctx.enter_context(tc.tile_pool(name="psq", bufs=2, space="PSUM"))
            pso = ctx.enter_context(tc.tile_pool(name="pso", bufs=4, space="PSUM"))
            psm = ctx.enter_context(tc.tile_pool(name="psm", bufs=2, space="PSUM"))

            live = {}

            def a_dma(n):
                """x tile DMA (SP) + per-token absmax/scale smalls (DVE)."""
                x_t = xpool.tile([128, D_IN], F32, tag="x")
                nc.sync.dma_start(x_t[:], x_d[n * 128:(n + 1) * 128, :])
                am = smpool.tile([128, 1], F32, tag="am")
                nc.vector.tensor_reduce(
                    am[:], x_t[:], axis=AX, op=op.max, apply_absolute_value=True
                )
                ram = smpool.tile([128, 1], F32, tag="ram")
                nc.vector.reciprocal(ram[:], am[:])
                scl = smpool.tile([128, 1], F32, tag="scl")
                nc.vector.tensor_scalar(scl[:], ram[:], QMAX, None, op0=op.mult)
                live[("x", n)] = x_t
                live[("am", n)] = am
                live[("scl", n)] = scl

            def a_quant(n):
                """single ACT pass: z16 = fp16(x*scl + 1536).  fp16 ulp in
                [1024,2048) is exactly 1, so the cast itself performs the RNE
                integer rounding - no separate subtract/cast pass needed.
                (GPSIMD measured 14.7us per 1024-elem op on HW - never
                offload bulk elementwise work to it.)"""
                x_t = live.pop(("x", n))
                scl = live.pop(("scl", n))
                xm = xmpool.tile([128, D_IN], FP16, tag="xm")
                nc.scalar.activation(xm[:], x_t[:], Copy, bias=MAGIC16, scale=scl[:])
                live[("q", n)] = xm

            def a_trans(n):
                """PE fp16 transposes + ACT drain (-1536 -> bf16 ints)."""
                q = live.pop(("q", n))
                ps_q = psq.tile([128, D_IN], FP16, tag="ps_q")
                for k in range(KT):
                    nc.tensor.transpose(
                        ps_q[:, k * 128:(k + 1) * 128],
                        q[:, k * 128:(k + 1) * 128],
                        ident[:],
                    )
                qT = qtpool.tile([128, D_IN], BF16, tag="qT")
                nc.scalar.activation(qT[:], ps_q[:], Copy, bias=-MAGIC16)
                live[("qT", n)] = qT

            def b_coef(n):
                am = live.pop(("am", n))
                coef = smpool.tile([128, 1], F32, tag="coef")
                nc.vector.tensor_scalar(coef[:], am[:], s127_bc[:], None, op0=op.mult)
                live[("coef", n)] = coef

            def b_mm_half(n, oh, qT=None):
                qT = qT if qT is not None else live[("qT", n)]
                ps = pso.tile([128, 512], F32, tag="ps")
                for k in range(KT):
                    nc.tensor.matmul(
                        ps[:], qT[:, k * 128:(k + 1) * 128],
                        tT[k][:, oh * 512:(oh + 1) * 512],
                        start=(k == 0), stop=(k == KT - 1),
                    )
                live[("ps", n, oh)] = ps

            def b_out_half(n, oh, out_sb):
                ps = live.pop(("ps", n, oh))
                coef = live[("coef", n)]
                nc.vector.tensor_scalar(
                    out_sb[:, oh * 512:(oh + 1) * 512], ps[:], coef[:], None,
                    op0=op.mult,
                )

            def b(n):
                """full tile: coef, both matmul halves, scales, out DMA."""
                b_coef(n)
                qT = live.pop(("qT", n))
                b_mm_half(n, 0, qT)
                b_mm_half(n, 1, qT)
                out_sb = outpool.tile([128, D_OUT], BF16, tag="osb")
                b_out_half(n, 0, out_sb)
                b_out_half(n, 1, out_sb)
                live.pop(("coef", n))
                nc.scalar.dma_start(o_d[n * 128:(n + 1) * 128, :], out_sb[:])

            # ---------------- weight phase + x ramp ------------------------
            with (
                tc.tile_pool(name="wpool", bufs=1) as wpool,
                tc.tile_pool(name="wabs", bufs=2) as wabs_pool,
                tc.tile_pool(name="ypool", bufs=4) as ypool,
            ):
                # DMA priority: wT first (it gates every matmul), split
                # across BOTH HWDGE rings to halve issue latency; x after.
                w_t = [wpool.tile([128, D_OUT], F32, name=f"w{k}", tag=f"w{k}") for k in range(KT)]
                for k in range(KT):
                    eng = nc.sync if k % 2 == 0 else nc.scalar
                    eng.dma_start(w_t[k][:], w_d[k * 128:(k + 1) * 128, :])
                a_dma(0)
                a_dma(1)
                a_dma(2)
                a_dma(3)
                a_dma(4)
                a_dma(5)

                # ramp tiles 0,1 quant+transpose while wT streams in
                a_quant(0)
                a_trans(0)
                a_quant(1)
                a_trans(1)

                # |wT| chunk sums (ACT even / DVE odd), then mean -> s, 1/s
                colsum = wpool.tile([128, KT], F32)
                for k in range(KT):
                    if k % 2 == 0:
                        wabs = wabs_pool.tile([128, D_OUT], F32, tag="wabs")
                        nc.scalar.activation(
                            wabs[:], w_t[k][:], Abs, accum_out=colsum[:, k:k + 1]
                        )
                    else:
                        nc.vector.tensor_reduce(
                            colsum[:, k:k + 1], w_t[k][:], axis=AX, op=op.add,
                            apply_absolute_value=True,
                        )
                colsum2 = wpool.tile([128, 1], F32)
                nc.vector.tensor_reduce(colsum2[:], colsum[:], axis=AX, op=op.add)

                ps_m1 = psm.tile([1, 2], F32, name="ps_m1", tag="ps_m")
                nc.tensor.matmul(ps_m1[0:1, 0:1], ones_col[:], colsum2[:])
                pair = wpool.tile([1, 2], F32)
                nc.scalar.activation(pair[:, 0:1], ps_m1[0:1, 0:1], Copy, scale=1.0 / (D_OUT * D_IN))
                nc.vector.reciprocal(pair[:, 1:2], pair[:, 0:1])
                ps_m2 = psm.tile([128, 2], F32, name="ps_m2", tag="ps_m")
                nc.tensor.matmul(ps_m2[:], ones_row[:], pair[:])
                nc.scalar.copy(bc2[:], ps_m2[:])
                nc.vector.tensor_scalar(s127_bc[:], bc2[:, 0:1], 1.0 / QMAX, None, op0=op.mult)

                # ternary-quantize wT in 512-col halves (oh-major) so the
                # first matmuls can start before the oh1 halves exist.
                def wq_half(k, oh):
                    sl = slice(oh * 512, (oh + 1) * 512)
                    y0 = ypool.tile([128, 512], F32, tag="y0")
                    nc.scalar.activation(y0[:], w_t[k][:, sl], Copy, scale=bc2[:, 1:2])
                    y1 = ypool.tile([128, 512], F32, tag="y1")
                    nc.vector.tensor_scalar(y1[:], y0[:], THR, -THR, op0=op.min, op1=op.max)
                    nc.vector.tensor_scalar(
                        tT[k][:, sl], y1[:], MAGIC, -MAGIC, op0=op.add, op1=op.add
                    )

                for k in range(KT):
                    wq_half(k, 0)
                # tiles 0,1: consume oh0 halves while oh1 halves are produced
                # (all oh1 writes are EMITTED before the oh1 matmuls so Tile
                # sees the dependency; execution still overlaps per-k.)
                b_coef(0)
                b_coef(1)
                qT0 = live.pop(("qT", 0))
                qT1 = live.pop(("qT", 1))
                b_mm_half(0, 0, qT0)
                wq_half(0, 1)
                wq_half(1, 1)
                a_quant(2)
                b_mm_half(1, 0, qT1)
                wq_half(2, 1)
                wq_half(3, 1)
                a_trans(2)
                wq_half(4, 1)
                wq_half(5, 1)
                a_quant(3)
                wq_half(6, 1)
                wq_half(7, 1)
                b_mm_half(0, 1, qT0)
                b_mm_half(1, 1, qT1)
                a_trans(3)

                osb0 = outpool.tile([128, D_OUT], BF16, tag="osb")
                b_out_half(0, 0, osb0)
                b_out_half(0, 1, osb0)
                live.pop(("coef", 0))
                nc.scalar.dma_start(o_d[0:128, :], osb0[:])
                osb1 = outpool.tile([128, D_OUT], BF16, tag="osb")
                b_out_half(1, 0, osb1)
                b_out_half(1, 1, osb1)
                live.pop(("coef", 1))
                nc.scalar.dma_start(o_d[128:256, :], osb1[:])

            a_quant(4)
            a_trans(4)

            for n in range(2, TILES):
                if n + 4 < TILES:
                    a_dma(n + 4)
                if n + 3 < TILES:
                    a_quant(n + 3)
                b(n)
                if n + 3 < TILES:
                    a_trans(n + 3)

    _split_multiwaits(nc)
    return nc


_NC_CACHE = None


def _get_nc():
    global _NC_CACHE
    if _NC_CACHE is None:
        _NC_CACHE = build_program()
    return _NC_CACHE


def kernel(x: np.ndarray, weight: np.ndarray, trace: bool = False):
    assert x.shape == (B, S, D_IN) and weight.shape == (D_OUT, D_IN)
    nc = _get_nc()
    xf = np.ascontiguousarray(x.reshape(TOKENS, D_IN), dtype=np.float32)
    wT = np.ascontiguousarray(weight.astype(np.float32, copy=False).T)
    in_maps = [
        {
            "x": xf[c * TOK_PER_CORE:(c + 1) * TOK_PER_CORE],
            "wT": wT,
        }
        for c in range(N_CORES)
    ]
    res = run_bass_kernel_spmd(nc, in_maps, core_ids=list(range(N_CORES)), trace=trace)
    kernel.last_results = res
    out = np.concatenate(
        [np.asarray(res.results[c]["out"]).astype(np.float32) for c in range(N_CORES)],
        axis=0,
    )
    return out.reshape(B, S, D_OUT)


kernel.last_results = None


# revision 22
# speedup vs baseline: 3.1452x; 1.0009x over previous
"""BitLinear (BitNet b1.58) Trainium2 kernel, 8-core data-parallel. v3

Per core (4096 tokens, weight replicated, fed host-transposed as wT):
    q  = round(x*scale)  integers in [-127,127]   -> exact in bf16
    t  = clip(round(W/s),-1,1) in {-1,0,1}        -> exact in bf16
    out = (q @ t.T) * (absmax*s/127) per token, stored bf16.

Engine plan per 128-token tile (steady state ~4.0us, PE-bound):
    SP   : x DMA in
    DVE  : absmax reduce, 1/absmax, scl, coef, both 512-col output scales
    ACT  : xm = x*scl + MAGIC (fp32 magic-number rounding), PSUM drain
    Pool : q = xm - MAGIC -> bf16 cast (SBUF->SBUF)
    PE   : 8 bf16 transposes (q -> qT via identity matmul, bf16 PSUM)
           + 16 bf16 matmuls (qT_k stationary, ternary tT moving)
    ACT ring: out DMA (bf16)

The weight phase overlaps the x ramp: wT streams in while tiles 0-1 are
quantized; ternary quantization runs in 512-col halves so the first
matmuls start as soon as the first halves exist; tiles 0/1 consume
oh0-halves of all k before any oh1-half is needed.
"""

import numpy as np

import concourse.bass as bass
import concourse.mybir as mybir
from concourse import tile, masks
from concourse.bass_utils import run_bass_kernel_spmd

F32 = mybir.dt.float32
BF16 = mybir.dt.bfloat16
FP16 = mybir.dt.float16

N_CORES = 8
B, S, D_IN, D_OUT = 4, 8192, 1024, 1024
TOKENS = B * S                     # 32768
TOK_PER_CORE = TOKENS // N_CORES   # 4096
TILES = TOK_PER_CORE // 128        # 32
KT = D_IN // 128                   # 8 contraction k-chunks

QMAX = 127.0
MAGIC = 12582912.0                     # 1.5 * 2**23 -> RNE integer rounding (fp32)
MAGIC16 = 1536.0                       # 1.5 * 2**10  -> RNE rounding via fp16 cast
THR = float(np.nextafter(np.float32(1.5), np.float32(0)))  # largest f32 < 1.5


def _split_multiwaits(nc):
    """walrus here encodes at most ONE sem wait per instruction; Tile's tail
    drain (and occasionally other insts) carry several.  Split extras into
    single-wait NOPs on the same engine, preserving order."""
    for f in nc.m.functions:
        for bb in f.blocks:
            insts = list(bb.instructions)
            if not any(
                i.sync_info and len(i.sync_info.on_wait) > 1 for i in insts
            ):
                continue
            new = []
            for ins in insts:
                si = ins.sync_info
                if si and len(si.on_wait) > 1:
                    waits = list(si.on_wait)
                    for j, w in enumerate(waits[:-1]):
                        nop = mybir.InstNoOp(
                            name=f"{ins.name}_wsp{j}", ins=[], outs=[]
                        )
                        nop.engine = ins.engine
                        nop.sync_info = mybir.SyncInfo(on_wait=[w], on_update=[])
                        new.append(nop)
                    ins.sync_info = mybir.SyncInfo(
                        on_wait=[waits[-1]], on_update=list(si.on_update)
                    )
                new.append(ins)
            bb.instructions = new


def build_program():
    nc = bass.Bass(trn_type="TRN2")
    x_d = nc.dram_tensor("x", [TOK_PER_CORE, D_IN], F32, kind="ExternalInput")
    w_d = nc.dram_tensor("wT", [D_IN, D_OUT], F32, kind="ExternalInput")
    o_d = nc.dram_tensor("out", [TOK_PER_CORE, D_OUT], BF16, kind="ExternalOutput")

    Copy = mybir.ActivationFunctionType.Copy
    Abs = mybir.ActivationFunctionType.Abs
    AX = mybir.AxisListType.X
    op = mybir.AluOpType

    with tile.TileContext(nc) as tc:
        from contextlib import ExitStack

        with ExitStack() as ctx:
            singles = ctx.enter_context(tc.tile_pool(name="singles", bufs=1))

            ident = singles.tile([128, 128], FP16)
            masks.make_identity(nc, ident[:])
            ones_col = singles.tile([128, 1], F32)
            nc.vector.memset(ones_col[:], 1.0)
            ones_row = singles.tile([1, 128], F32)
            nc.vector.memset(ones_row[:], 1.0)
            bc2 = singles.tile([128, 2], F32)    # [s, 1/s] broadcast to 128 parts
            s127_bc = singles.tile([128, 1], F32)  # s/127 broadcast

            tT = [singles.tile([128, D_OUT], BF16, name=f"tT{k}", tag=f"tT{k}") for k in range(KT)]

            xpool = ctx.enter_context(tc.tile_pool(name="xpool", bufs=8))
            xmpool = ctx.enter_context(tc.tile_pool(name="xmpool", bufs=3))
            qtpool = ctx.enter_context(tc.tile_pool(name="qtpool", bufs=6))
            outpool = ctx.enter_context(tc.tile_pool(name="outpool", bufs=3))
            smpool = ctx.enter_context(tc.tile_pool(name="smpool", bufs=12))
            psq = ctx.enter_context(tc.tile_pool(name="psq", bufs=2, space="PSUM"))
            pso = ctx.enter_context(tc.tile_pool(name="pso", bufs=4, space="PSUM"))
            psm = ctx.enter_context(tc.tile_pool(name="psm", bufs=2, space="PSUM"))

            live = {}

            def a_dma_issue(n):
                """x tile DMA issue only (SP ring)."""
                x_t = xpool.tile([128, D_IN], F32, tag="x")
                nc.sync.dma_start(x_t[:], x_d[n * 128:(n + 1) * 128, :])
                live[("x", n)] = x_t

            def a_stats(n):
                """per-token absmax/scale smalls (DVE)."""
                x_t = live[("x", n)]
                am = smpool.tile([128, 1], F32, tag="am")
                nc.vector.tensor_reduce(
                    am[:], x_t[:], axis=AX, op=op.max, apply_absolute_value=True
                )
                ram = smpool.tile([128, 1], F32, tag="ram")
                nc.vector.reciprocal(ram[:], am[:])
                scl = smpool.tile([128, 1], F32, tag="scl")
                nc.vector.tensor_scalar(scl[:], ram[:], QMAX, None, op0=op.mult)
                live[("am", n)] = am
                live[("scl", n)] = scl

            def a_dma(n):
                a_dma_issue(n)
                a_stats(n)

            def a_quant(n):
                """single ACT pass: z16 = fp16(x*scl + 1536).  fp16 ulp in
                [1024,2048) is exactly 1, so the cast itself performs the RNE
                integer rounding - no separate subtract/cast pass needed.
                (GPSIMD measured 14.7us per 1024-elem op on HW - never
                offload bulk elementwise work to it.)"""
                x_t = live.pop(("x", n))
                scl = live.pop(("scl", n))
                xm = xmpool.tile([128, D_IN], FP16, tag="xm")
                nc.scalar.activation(xm[:], x_t[:], Copy, bias=MAGIC16, scale=scl[:])
                live[("q", n)] = xm

            def a_trans(n):
                """PE fp16 transposes + ACT drain (-1536 -> bf16 ints)."""
                q = live.pop(("q", n))
                ps_q = psq.tile([128, D_IN], FP16, tag="ps_q")
                for k in range(KT):
                    nc.tensor.transpose(
                        ps_q[:, k * 128:(k + 1) * 128],
                        q[:, k * 128:(k + 1) * 128],
                        ident[:],
                    )
                qT = qtpool.tile([128, D_IN], BF16, tag="qT")
                nc.scalar.activation(qT[:], ps_q[:], Copy, bias=-MAGIC16)
                live[("qT", n)] = qT

            def b_coef(n):
                am = live.pop(("am", n))
                coef = smpool.tile([128, 1], F32, tag="coef")
                nc.vector.tensor_scalar(coef[:], am[:], s127_bc[:], None, op0=op.mult)
                live[("coef", n)] = coef

            def b_mm_half(n, oh, qT=None):
                qT = qT if qT is not None else live[("qT", n)]
                ps = pso.tile([128, 512], F32, tag="ps")
                for k in range(KT):
                    nc.tensor.matmul(
                        ps[:], qT[:, k * 128:(k + 1) * 128],
                        tT[k][:, oh * 512:(oh + 1) * 512],
                        start=(k == 0), stop=(k == KT - 1),
                    )
                live[("ps", n, oh)] = ps

            def b_out_half(n, oh, out_sb):
                ps = live.pop(("ps", n, oh))
                coef = live[("coef", n)]
                nc.vector.tensor_scalar(
                    out_sb[:, oh * 512:(oh + 1) * 512], ps[:], coef[:], None,
                    op0=op.mult,
                )

            def b(n, split_out=False):
                """full tile: coef, both matmul halves, scales, out DMA.
                split_out: scale halves on ACT+DVE in parallel and DMA each
                half as soon as it's ready (shorter tail for the last tile)."""
                b_coef(n)
                qT = live.pop(("qT", n))
                b_mm_half(n, 0, qT)
                b_mm_half(n, 1, qT)
                out_sb = outpool.tile([128, D_OUT], BF16, tag="osb")
                if split_out:
                    ps0 = live.pop(("ps", n, 0))
                    ps1 = live.pop(("ps", n, 1))
                    coef = live.pop(("coef", n))
                    nc.scalar.activation(out_sb[:, 0:512], ps0[:], Copy, scale=coef[:])
                    nc.scalar.dma_start(
                        o_d[n * 128:(n + 1) * 128, 0:512], out_sb[:, 0:512]
                    )
                    nc.vector.tensor_scalar(
                        out_sb[:, 512:1024], ps1[:], coef[:], None, op0=op.mult
                    )
                    nc.sync.dma_start(
                        o_d[n * 128:(n + 1) * 128, 512:1024], out_sb[:, 512:1024]
                    )
                    return
                b_out_half(n, 0, out_sb)
                b_out_half(n, 1, out_sb)
                live.pop(("coef", n))
                nc.scalar.dma_start(o_d[n * 128:(n + 1) * 128, :], out_sb[:])

            # ---------------- weight phase + x ramp ------------------------
            with (
                tc.tile_pool(name="wpool", bufs=1) as wpool,
                tc.tile_pool(name="wabs", bufs=2) as wabs_pool,
                tc.tile_pool(name="ypool", bufs=4) as ypool,
            ):
                # DMA priority: wT almost entirely first (it gates every
                # matmul - the chip HBM is saturated during the head, so
                # every early x byte delays s 1:1), x0 squeezed mid-stream
                # so its quant pipeline overlaps the w tail, x1..x5 after.
                # Rings alternate to halve issue latency.
                w_t = [wpool.tile([128, D_OUT], F32, name=f"w{k}", tag=f"w{k}") for k in range(KT)]
                for k in range(5):
                    eng = nc.sync if k % 2 == 0 else nc.scalar
                    eng.dma_start(w_t[k][:], w_d[k * 128:(k + 1) * 128, :])
                a_dma_issue(0)
                for k in range(5, KT):
                    eng = nc.sync if k % 2 == 0 else nc.scalar
                    eng.dma_start(w_t[k][:], w_d[k * 128:(k + 1) * 128, :])
                for n in range(1, 6):
                    a_dma_issue(n)

                # |wT| chunk sums (ACT even / DVE odd) in arrival order,
                # x0/x1 stats slotted so they don't block later colsums.
                colsum = wpool.tile([128, KT], F32)

                def cs(k):
                    if k % 2 == 0:
                        wabs = wabs_pool.tile([128, D_OUT], F32, tag="wabs")
                        nc.scalar.activation(
                            wabs[:], w_t[k][:], Abs, accum_out=colsum[:, k:k + 1]
                        )
                    else:
                        nc.vector.tensor_reduce(
                            colsum[:, k:k + 1], w_t[k][:], axis=AX, op=op.add,
                            apply_absolute_value=True,
                        )

                for k in range(5):
                    cs(k)
                a_stats(0)
                cs(5)
                cs(6)
                cs(7)
                colsum2 = wpool.tile([128, 1], F32)
                nc.vector.tensor_reduce(colsum2[:], colsum[:], axis=AX, op=op.add)

                # tile 0 quant+transpose overlaps the w tail
                a_quant(0)
                a_trans(0)

                ps_m1 = psm.tile([1, 2], F32, name="ps_m1", tag="ps_m")
                nc.tensor.matmul(ps_m1[0:1, 0:1], ones_col[:], colsum2[:])
                pair = wpool.tile([1, 2], F32)
                nc.scalar.activation(pair[:, 0:1], ps_m1[0:1, 0:1], Copy, scale=1.0 / (D_OUT * D_IN))
                nc.vector.reciprocal(pair[:, 1:2], pair[:, 0:1])
                ps_m2 = psm.tile([128, 2], F32, name="ps_m2", tag="ps_m")
                nc.tensor.matmul(ps_m2[:], ones_row[:], pair[:])
                nc.scalar.copy(bc2[:], ps_m2[:])
                nc.vector.tensor_scalar(s127_bc[:], bc2[:, 0:1], 1.0 / QMAX, None, op0=op.mult)

                a_stats(1)
                a_quant(1)
                a_trans(1)

                # ternary-quantize wT in 512-col halves (oh-major) so the
                # first matmuls can start before the oh1 halves exist.
                def wq_half(k, oh):
                    sl = slice(oh * 512, (oh + 1) * 512)
                    y0 = ypool.tile([128, 512], F32, tag="y0")
                    nc.scalar.activation(y0[:], w_t[k][:, sl], Copy, scale=bc2[:, 1:2])
                    y1 = ypool.tile([128, 512], F32, tag="y1")
                    nc.vector.tensor_scalar(y1[:], y0[:], THR, -THR, op0=op.min, op1=op.max)
                    nc.vector.tensor_scalar(
                        tT[k][:, sl], y1[:], MAGIC, -MAGIC, op0=op.add, op1=op.add
                    )

                for k in range(KT):
                    wq_half(k, 0)
                # tiles 0,1: consume oh0 halves while oh1 halves are produced
                # (all oh1 writes are EMITTED before the oh1 matmuls so Tile
                # sees the dependency; execution still overlaps per-k.)
                b_coef(0)
                b_coef(1)
                qT0 = live.pop(("qT", 0))
                qT1 = live.pop(("qT", 1))
                b_mm_half(0, 0, qT0)
                wq_half(0, 1)
                wq_half(1, 1)
                a_stats(2)
                a_quant(2)
                b_mm_half(1, 0, qT1)
                wq_half(2, 1)
                wq_half(3, 1)
                a_trans(2)
                wq_half(4, 1)
                wq_half(5, 1)
                a_stats(3)
                a_quant(3)
                wq_half(6, 1)
                wq_half(7, 1)
                b_mm_half(0, 1, qT0)
                b_mm_half(1, 1, qT1)
                a_trans(3)

                osb0 = outpool.tile([128, D_OUT], BF16, tag="osb")
                b_out_half(0, 0, osb0)
                b_out_half(0, 1, osb0)
                live.pop(("coef", 0))
                nc.scalar.dma_start(o_d[0:128, :], osb0[:])
                osb1 = outpool.tile([128, D_OUT], BF16, tag="osb")
                b_out_half(1, 0, osb1)
                b_out_half(1, 1, osb1)
                live.pop(("coef", 1))
                nc.scalar.dma_start(o_d[128:256, :], osb1[:])

            a_stats(4)
            a_quant(4)
            a_trans(4)
            a_stats(5)

            for n in range(2, TILES):
                if n + 4 < TILES:
                    a_dma(n + 4)
                if n + 3 < TILES:
                    a_quant(n + 3)
                b(n, split_out=(n == TILES - 1))
                if n + 3 < TILES:
                    a_trans(n + 3)

    _split_multiwaits(nc)
    return nc


_NC_CACHE = None


def _get_nc():
    global _NC_CACHE
    if _NC_CACHE is None:
        _NC_CACHE = build_program()
    return _NC_CACHE


def kernel(x: np.ndarray, weight: np.ndarray, trace: bool = False):
    assert x.shape == (B, S, D_IN) and weight.shape == (D_OUT, D_IN)
    nc = _get_nc()
    xf = np.ascontiguousarray(x.reshape(TOKENS, D_IN), dtype=np.float32)
    wT = np.ascontiguousarray(weight.astype(np.float32, copy=False).T)
    in_maps = [
        {
            "x": xf[c * TOK_PER_CORE:(c + 1) * TOK_PER_CORE],
            "wT": wT,
        }
        for c in range(N_CORES)
    ]
    res = run_bass_kernel_spmd(nc, in_maps, core_ids=list(range(N_CORES)), trace=trace)
    kernel.last_results = res
    out = np.concatenate(
        [np.asarray(res.results[c]["out"]).astype(np.float32) for c in range(N_CORES)],
        axis=0,
    )
    return out.reshape(B, S, D_OUT)


kernel.last_results = None
